# revision 10
# baseline (speedup 1.0000x reference)
"""Causal self-attention Bass/Tile kernel for Trainium2, 8 NeuronCores.

Problem: B=4, T=2048, C=1024, NH=16, HD=64.
  q/k/v = x @ W{q,k,v}; att = softmax(causal(q k^T / 8)); y = (att v) @ Wp

Sharding (8 cores): batch (4-way) x head-group (2-way tensor parallel).
Core c handles batch b=c//2 and global heads g*8..g*8+7 where g=c%2.
Each core computes a partial projection y_part = y_heads_local @ Wp[rows]
and the host unshards by summing the two partial outputs per batch.

Per-core kernel (all T=2048 tokens, 8 heads, head_dim 64):
  Phase A: qT/kT = (x W)^T stored [d, t] (bf16), v stored [t, d] (bf16,
           augmented with a ones column so P^T@V also yields the softmax
           denominator l in PSUM row 64).
  Phase B: per head, per 512-token query tile j: transposed score tiles
           S^T [s:128, t:512] on PE (bf16), exp(S/8) on ACT (PSUM->SBUF),
           causal mask via GPSIMD affine_select (fill 0 post-exp),
           P^T@[V|1] on PE accumulating unnormalized out^T [65, t] in PSUM.
  Phase C: l -> 1/l (DVE reciprocal), broadcast over d via PE outer
           product with ones, normalize out^T in SBUF (DVE multiply).
  Phase D: y_part[t, c] = sum_u ylocT[u, t] * Wp[u, c] on PE (f32r).

Matmul dtypes: projections in float32r (fp32 bits, fast PE mode),
attention in bf16 with fp32 PSUM accumulation.
"""

import numpy as np

B, T, C, NH, HD = 4, 2048, 1024, 16, 64
G = 512          # local head dims per core (8 heads x 64)
P = 128
NT = 4           # t tiles of 512
NT128 = 16       # t tiles of 128
NPAIR = 4        # local head pairs
TT = 512

_CACHE = {}


def _build_nc():
    import concourse.tile as tile
    from concourse import bacc, mybir

    f32 = mybir.dt.float32
    f32r = mybir.dt.float32r
    bf16 = mybir.dt.bfloat16

    nc = bacc.Bacc("TRN2", target_bir_lowering=False, debug=False)

    xT = nc.dram_tensor("xt", [C, T], f32r, kind="ExternalInput")
    wq = nc.dram_tensor("wq", [C, G], f32r, kind="ExternalInput")
    wk = nc.dram_tensor("wk", [C, G], f32r, kind="ExternalInput")
    wv = nc.dram_tensor("wv", [C, G], f32r, kind="ExternalInput")
    wp = nc.dram_tensor("wp", [G, C], f32r, kind="ExternalInput")
    sel = nc.dram_tensor("sel", [2, P], f32, kind="ExternalInput")
    y = nc.dram_tensor("y", [T, C], f32, kind="ExternalOutput")

    xT_v = xT.rearrange("(co p) t -> p co t", p=P)      # [128, 8, 2048]
    wq_v = wq.rearrange("(co p) g -> p co g", p=P)      # [128, 8, 512]
    wk_v = wk.rearrange("(co p) g -> p co g", p=P)
    wv_v = wv.rearrange("(co p) g -> p co g", p=P)
    wp_v = wp.rearrange("(uo p) c -> p uo c", p=P)      # [128, 4, 1024]
    y_v = y.rearrange("(to p) c -> p to c", p=P)        # [128, 16, 1024]

    with tile.TileContext(nc) as tc:
        with (
            tc.tile_pool(name="singles", bufs=1) as singles,
            tc.tile_pool(name="xpool", bufs=2) as xpool,
            tc.tile_pool(name="wqk", bufs=2) as wqk_pool,
            tc.tile_pool(name="wvp", bufs=3) as wv_pool,
            tc.tile_pool(name="wpp", bufs=2) as wp_pool,
            tc.tile_pool(name="expst", bufs=2) as epool,
            tc.tile_pool(name="bcast", bufs=1) as bpool,
            tc.tile_pool(name="rf", bufs=2) as rfpool,
            tc.tile_pool(name="ystage", bufs=2) as ypool,
            tc.tile_pool(name="psA", bufs=4, space="PSUM") as psA,
            tc.tile_pool(name="psS", bufs=2, space="PSUM") as psS,
        ):
            # persistent tensors
            qT = singles.tile([P, NPAIR, T], bf16, name="qT", tag="qT")
            kT = singles.tile([P, NPAIR, T], bf16, name="kT", tag="kT")
            v_sb = singles.tile([P, NT128, 8, 66], bf16, name="v_sb", tag="v_sb")
            ylocT = singles.tile([P, NPAIR, T], f32r, name="ylocT", tag="ylocT")
            # l for (h, j) lives at partition 32*j, free slot h (DVE copies
            # out of PSUM row 64 may only target partitions 0/32/64/96)
            lq = singles.tile([P, 8, TT], f32, name="lq", tag="lq")
            # l8/recip8: pair pr at partitions {32pr, 32pr+1}
            l8 = singles.tile([P, NT, TT], f32, name="l8", tag="l8")
            recip8 = singles.tile([P, NT, TT], f32, name="recip8", tag="recip8")
            sel_sb = singles.tile([2, P], f32, name="sel_sb", tag="sel_sb")

            nc.vector.memset(v_sb[:, :, :, 64:65], 1.0)
            nc.vector.memset(l8[:], 1.0)
            nc.sync.dma_start(sel_sb[:], sel[:])

            # ---------------- Phase A: projections ----------------
            CH = 256  # t chunk
            for j in range(8):
                xt = xpool.tile([P, 8, CH], f32r, name="xt", tag="xt")
                nc.sync.dma_start(xt[:], xT_v[:, :, j * CH:(j + 1) * CH])
                
                # qT, kT: [d,t] via lhsT=W[c,d-128] rhs=xT[c,t-256]
                for dg in range(NPAIR):
                    for w_view, dstT in ((wq_v, qT), (wk_v, kT)):
                        ps = psA.tile([P, CH], f32, name="ps_qk", tag="psA")
                        for co in range(8):
                            wt = wqk_pool.tile([P, P], f32r, name="wt", tag="wt")
                            nc.sync.dma_start(
                                wt[:], w_view[:, co, dg * P:(dg + 1) * P])
                            nc.tensor.matmul(
                                ps[:], wt[:], xt[:, co, :],
                                start=(co == 0), stop=(co == 7))
                        nc.vector.tensor_copy(
                            out=dstT[:, dg, j * CH:(j + 1) * CH], in_=ps[:])

                # v: [t,g] via lhsT=xT[c,t-128] rhs=Wv[c,g-512]
                for tq in range(2):
                    t128 = 2 * j + tq
                    ps = psA.tile([P, G], f32, name="ps_v", tag="psA")
                    for co in range(8):
                        wvt = wv_pool.tile([P, G], f32r, name="wvt", tag="wvt")
                        nc.sync.dma_start(wvt[:], wv_v[:, co, :])
                        nc.tensor.matmul(
                            ps[:], xt[:, co, tq * P:(tq + 1) * P],
                            wvt[:],
                            start=(co == 0), stop=(co == 7))
                    nc.vector.tensor_copy(
                        out=v_sb[:, t128, :, 0:64],
                        in_=ps.rearrange("p (h d) -> p h d", h=8))

            # ---------------- Phase B: attention ----------------
            for pr in range(NPAIR):
                for j in range(NT):
                    ns = 4 * (j + 1)  # s tiles of 128 in causal prefix
                    for hi in range(2):
                        h = 2 * pr + hi
                        hp = 64 * hi
                        expst = epool.tile(
                            [P, NT128, TT], bf16, name="expst", tag="expst")
                        # scores^T + exp, chunks of 2 s-tiles
                        for ck in range(ns // 2):
                            ps_s = psS.tile([P, 2, TT], f32, name="ps_s", tag="psS")
                            for u in range(2):
                                so = 2 * ck + u
                                nc.tensor.matmul(
                                    ps_s[:, u, :],
                                    kT[hp:hp + 64, pr, so * P:(so + 1) * P],
                                    qT[hp:hp + 64, pr, j * TT:(j + 1) * TT],
                                    start=True, stop=True)
                            nc.scalar.activation(
                                out=expst[:, 2 * ck:2 * ck + 2, :],
                                in_=ps_s[:],
                                func=mybir.ActivationFunctionType.Exp,
                                scale=0.125)
                        # causal mask on diagonal 4 s-tiles (s > t -> 0)
                        nc.gpsimd.affine_select(
                            out=expst[:, 4 * j:4 * j + 4, :],
                            in_=expst[:, 4 * j:4 * j + 4, :],
                            pattern=[[-P, 4], [1, TT]],
                            compare_op=mybir.AluOpType.is_ge,
                            fill=0.0,
                            base=0,
                            channel_multiplier=-1)
                        # P^T @ [v | 1] accumulating out^T (65 rows)
                        ps_o = psA.tile([P, TT], f32, name="ps_o", tag="psA")
                        for so in range(ns):
                            nc.tensor.matmul(
                                ps_o[0:65, :],
                                v_sb[:, so, h, 0:65],
                                expst[:, so, :],
                                start=(so == 0), stop=(so == ns - 1))
                        nc.vector.tensor_copy(
                            out=ylocT[hp:hp + 64, pr, j * TT:(j + 1) * TT],
                            in_=ps_o[0:64, :])
                        nc.vector.tensor_copy(
                            out=lq[32 * j:32 * j + 1, h, :],
                            in_=ps_o[64:65, :])

            # ---------------- Phase C: normalize ----------------
            # gather l (partition 32j, slot h) -> l8 (partition 32pr+hi, slot j)
            for j in range(NT):
                for pr in range(NPAIR):
                    nc.sync.dma_start(
                        out=l8[32 * pr:32 * pr + 2, j, :],
                        in_=lq[32 * j:32 * j + 1, 2 * pr:2 * pr + 2, :])
            nc.vector.reciprocal(out=recip8[:], in_=l8[:])
            # broadcast 1/l over the pair's 128 head dims via selector matmul:
            # bcast[m, t] = sel[0, m] * recip_h0[t] + sel[1, m] * recip_h1[t]
            for pr in range(NPAIR):
                for hc in range(2):  # t halves of 1024
                    rf = rfpool.tile([2, 2, TT], f32, name="rf", tag="rf")
                    nc.sync.dma_start(
                        out=rf[:],
                        in_=recip8[32 * pr:32 * pr + 2, 2 * hc:2 * hc + 2, :])
                    ps_b = psS.tile([P, 2, TT], f32, name="ps_b", tag="psS")
                    for u in range(2):
                        nc.tensor.matmul(
                            ps_b[:, u, :], sel_sb[:], rf[:, u, :],
                            start=True, stop=True)
                    bc = bpool.tile([P, 2, TT], f32, name="bc", tag="bc")
                    nc.vector.tensor_copy(out=bc[:], in_=ps_b[:])
                    yv = ylocT[:, pr, hc * 1024:(hc + 1) * 1024]
                    nc.vector.tensor_tensor(
                        out=yv.rearrange("p (a b) -> p a b", a=2),
                        in0=yv.rearrange("p (a b) -> p a b", a=2),
                        in1=bc[:],
                        op=mybir.AluOpType.mult)

            # ---------------- Phase D: output projection ----------------
            for cn in range(2):
                for tg in range(4):
                    for tq in range(4):
                        t128 = 4 * tg + tq
                        ps_y = psA.tile([P, TT], f32, name="ps_y", tag="psA")
                        for uo in range(4):
                            wpt = wp_pool.tile([P, TT], f32r, name="wpt", tag="wpt")
                            nc.sync.dma_start(
                                wpt[:],
                                wp_v[:, uo, cn * TT:(cn + 1) * TT])
                            nc.tensor.matmul(
                                ps_y[:],
                                ylocT[:, uo, t128 * P:(t128 + 1) * P],
                                wpt[:],
                                start=(uo == 0), stop=(uo == 3))
                        yst = ypool.tile([P, TT], f32, name="yst", tag="yst")
                        nc.scalar.copy(out=yst[:], in_=ps_y[:])
                        nc.sync.dma_start(
                            out=y_v[:, t128, cn * TT:(cn + 1) * TT],
                            in_=yst[:])

    nc.finalize()
    return nc


def _get_nc():
    if "nc" not in _CACHE:
        _CACHE["nc"] = _build_nc()
    return _CACHE["nc"]


def _sel_array():
    sel = np.zeros((2, P), np.float32)
    sel[0, 0:64] = 1.0
    sel[1, 64:128] = 1.0
    return sel


def shard_inputs(x, Wq, Wk, Wv, Wp):
    """Build the 8 per-core input maps."""
    x = np.asarray(x, np.float32)
    in_maps = []
    for c in range(8):
        b, g = c // 2, c % 2
        sl = slice(g * G, (g + 1) * G)
        in_maps.append({
            "xt": np.ascontiguousarray(x[b].T),
            "wq": np.ascontiguousarray(np.asarray(Wq, np.float32)[:, sl]),
            "wk": np.ascontiguousarray(np.asarray(Wk, np.float32)[:, sl]),
            "wv": np.ascontiguousarray(np.asarray(Wv, np.float32)[:, sl]),
            "wp": np.ascontiguousarray(np.asarray(Wp, np.float32)[sl, :]),
            "sel": _sel_array(),
        })
    return in_maps


def unshard_outputs(results):
    """results: list of 8 dicts with 'y' [T, C] partials -> [B, T, C]."""
    out = np.empty((B, T, C), np.float32)
    for b in range(B):
        out[b] = results[2 * b]["y"] + results[2 * b + 1]["y"]
    return out


def kernel(**inputs):
    from concourse import bass_utils
    nc = _get_nc()
    in_maps = shard_inputs(**inputs)
    res = bass_utils.run_bass_kernel_spmd(nc, in_maps, core_ids=list(range(8)))
    return unshard_outputs(res.results)
